# revision 6
# baseline (speedup 1.0000x reference)
"""Bahdanau additive attention on 8 Trainium2 NeuronCores — sine-expansion
kernel (v4, all-bf16 datapath, host-side transposes).

tanh(s) ~= c*s + sum_k beta_k sin(omega_k s)   (K=5 free-frequency fit)
sin(w(a+b)) = sin(wa)cos(wb)+cos(wa)sin(wb)  ->  score = 2K+1 rank-128 matmuls.

Per core: batch b=c//2, decoder rows (c%2)*128..+128.
  args ap[u, e|t] = W^T X + bias   (bf16 matmuls from host-pretransposed X^T)
  per k: n = i32(ap*u_k + 1/8)  [round-to-nearest, DVE]
         g = ap*u_k - n         [fused stt, DVE]  in [-0.625, 0.375]
         sinf = Sin(2pi g), cosf = Sin(2pi g + pi/2)   [ACT, bf16 out]
         sdec = dec-side features * (v*beta_k)         [DVE, bf16]
         score += sdec^T @ encf  (2 PE matmuls)
  linear term c*sum_u v_u a[e,u] via one f32r matmul (c*b cancels in softmax).
  Split exp (two halves) overlaps esc transposes + ctx accumulation with the
  second half; attn = esc*rinv; ctx = (escT @ enc)*rinv.
"""

import numpy as np

B, TE, TD, DE, U = 4, 512, 256, 512, 128
N_CORES = 8
ROWS = 128  # decoder rows per core

K = 5
C_LIN = 0.18337023548468817
OMEGA = [0.5781886445225611, 1.1653011938962863, 1.768937576812589,
         3.5219593878693645, 2.4965661831194406]
BETA = [0.5607026898029832, 0.19448055206553636, 0.07922678005074131,
        0.009694562725872236, 0.03368730304595286]

_CACHE = {}


def _build_program():
    from contextlib import ExitStack

    import concourse.bacc as bacc
    import concourse.tile as tile
    from concourse import mybir
    from concourse.masks import make_identity

    f32 = mybir.dt.float32
    f32r = mybir.dt.float32r
    i32 = mybir.dt.int32
    bf16 = mybir.dt.bfloat16
    AF = mybir.ActivationFunctionType
    ALU = mybir.AluOpType

    TWO_PI = float(2 * np.pi)
    HALF_PI = float(np.pi / 2)
    UK = [float(w / (2 * np.pi)) for w in OMEGA]

    nc = bacc.Bacc("TRN2", target_bir_lowering=False, debug=False)

    encT_dr = nc.dram_tensor("encT", [DE, TE], bf16, kind="ExternalInput")
    decT_dr = nc.dram_tensor("decT", [DE, ROWS], bf16, kind="ExternalInput")
    enc_d = nc.dram_tensor("enc", [TE, DE], bf16, kind="ExternalInput")
    w1_d = nc.dram_tensor("w1", [DE, U], bf16, kind="ExternalInput")
    w2_d = nc.dram_tensor("w2", [DE, U], bf16, kind="ExternalInput")
    vpack_d = nc.dram_tensor("vpack", [U, 3], f32, kind="ExternalInput")
    ctx_d = nc.dram_tensor("ctx", [ROWS, DE], f32, kind="ExternalOutput")
    attn_d = nc.dram_tensor("attn", [ROWS, TE], f32, kind="ExternalOutput")

    ND = DE // 128  # contraction blocks
    NT = TE // 128  # te chunks
    APW = TE + ROWS  # concat width: enc args | dec args

    with tile.TileContext(nc) as tc, ExitStack() as ctx:
        const = ctx.enter_context(tc.tile_pool(name="const", bufs=1))
        work = ctx.enter_context(tc.tile_pool(name="work", bufs=3))
        ps_p = ctx.enter_context(tc.tile_pool(name="ps_p", bufs=1, space="PSUM"))
        ps_s = ctx.enter_context(tc.tile_pool(name="ps_s", bufs=1, space="PSUM"))

        # ---- input DMAs: weights + encT first (they gate the projections) ----
        encT = [
            const.tile([128, TE], bf16, tag=f"encT_{d}", name=f"encT_{d}")
            for d in range(ND)
        ]
        decT = const.tile([128, ND, 128], bf16, tag="decT")
        enc_nat = const.tile([128, NT, DE], bf16, tag="encN")
        w1_sb = const.tile([128, ND, U], bf16, tag="w1")
        w2_sb = const.tile([128, ND, U], bf16, tag="w2")
        vpack = const.tile([U, 3], f32, tag="vpack")

        nc.sync.dma_start(out=encT[0], in_=encT_dr[0:128, :])
        nc.scalar.dma_start(out=w1_sb,
                            in_=w1_d.rearrange("(k p) u -> p k u", p=128))
        nc.sync.dma_start(out=encT[1], in_=encT_dr[128:256, :])
        nc.scalar.dma_start(out=w2_sb,
                            in_=w2_d.rearrange("(k p) u -> p k u", p=128))
        nc.sync.dma_start(out=vpack, in_=vpack_d[:, :])
        nc.scalar.dma_start(out=decT,
                            in_=decT_dr.rearrange("(k p) t -> p k t", p=128))
        nc.sync.dma_start(out=encT[2], in_=encT_dr[256:384, :])
        nc.scalar.dma_start(out=encT[3], in_=encT_dr[384:512, :])
        nc.scalar.dma_start(out=enc_nat,
                            in_=enc_d.rearrange("(c p) d -> p c d", p=128))
        v_sb = vpack[:, 0:1]
        w1b_sb = vpack[:, 1:2]
        w2b_sb = vpack[:, 2:3]

        halfpi = const.tile([128, 1], f32, tag="halfpi")
        nc.vector.memset(halfpi, HALF_PI)
        # per-partition biases for the direct k=0 features from PSUM:
        # sin(w0*(x+b)) = Sin(x, scale=w0, bias=w0*b); cos adds pi/2
        b0 = const.tile([U, 4], f32, tag="b0")
        nc.vector.tensor_scalar(b0[:, 0:1], w1b_sb, OMEGA[0], None,
                                op0=ALU.mult)
        nc.vector.tensor_scalar(b0[:, 1:2], w1b_sb, OMEGA[0], HALF_PI,
                                op0=ALU.mult, op1=ALU.add)
        nc.vector.tensor_scalar(b0[:, 2:3], w2b_sb, OMEGA[0], None,
                                op0=ALU.mult)
        nc.vector.tensor_scalar(b0[:, 3:4], w2b_sb, OMEGA[0], HALF_PI,
                                op0=ALU.mult, op1=ALU.add)
        ident_b = const.tile([128, 128], bf16, tag="ident_b")
        idf = const.tile([128, 128], f32, tag="ident_f")
        make_identity(nc, idf)
        nc.vector.tensor_copy(ident_b, idf)
        vb = const.tile([U, K], f32, tag="vb")
        for k in range(K):
            nc.vector.tensor_scalar(vb[:, k:k + 1], v_sb, BETA[k], None,
                                    op0=ALU.mult)
        ones = const.tile([U, ROWS], f32, tag="ones")
        nc.vector.memset(ones, 1.0)
        cvrep = const.tile([U, ROWS], f32r, tag="cvrep")
        nc.vector.tensor_scalar(cvrep, ones, v_sb, C_LIN, op0=ALU.mult,
                                op1=ALU.mult)

        # ---- projections (bf16 matmuls, f32 PSUM) ----
        # PE order follows DMA arrival: ep d0,d1 -> dp (decT+w2 land in the
        # gap before encT2/3) -> ep d2,d3.
        ap = const.tile([U, APW], f32, tag="ap")
        ep = ps_p.tile([U, TE], f32, tag="ep", name="ep")
        dp = ps_p.tile([U, ROWS], f32, tag="dp", name="dp")
        for d in (0, 1):
            nc.tensor.matmul(ep, w1_sb[:, d, :], encT[d],
                             start=(d == 0), stop=False, skip_group_check=True)
        for d in range(ND):
            nc.tensor.matmul(dp, w2_sb[:, d, :], decT[:, d, :],
                             start=(d == 0), stop=(d == ND - 1),
                             skip_group_check=True)
        for d in (2, 3):
            nc.tensor.matmul(ep, w1_sb[:, d, :], encT[d],
                             start=False, stop=(d == ND - 1),
                             skip_group_check=True)
        nc.vector.tensor_scalar_add(ap[:, TE:], dp, w2b_sb)
        nc.vector.tensor_scalar_add(ap[:, 0:TE], ep, w1b_sb)

        # ---- score: linear term + per-frequency terms ----
        score = ps_s.tile([ROWS, TE], f32, tag="score", name="score")
        ap_r = const.tile([U, TE], f32r, tag="ap_r")

        feat = const.tile([U, 2 * K, APW], bf16, tag="feat")
        sdec = const.tile([U, 2 * K, ROWS], bf16, tag="sdec")

        def chain(k):
            n_k = work.tile([U, APW], i32, tag="n", name=f"n{k}")
            nc.vector.tensor_scalar(n_k, ap, UK[k], 0.125, op0=ALU.mult,
                                    op1=ALU.add)
            g_k = work.tile([U, APW], f32, tag="g", name=f"g{k}")
            nc.vector.scalar_tensor_tensor(g_k, ap, UK[k], n_k,
                                           op0=ALU.mult, op1=ALU.subtract)
            return g_k

        def sins(k, g_k):
            nc.scalar.activation(feat[:, 2 * k, :], g_k, AF.Sin, scale=TWO_PI)
            nc.scalar.activation(feat[:, 2 * k + 1, :], g_k, AF.Sin,
                                 scale=TWO_PI, bias=halfpi)

        def score_terms(k, last=False):
            nc.vector.tensor_scalar(sdec[:, 2 * k:2 * k + 2, :],
                                    feat[:, 2 * k:2 * k + 2, TE:],
                                    vb[:, k:k + 1], None, op0=ALU.mult)
            nc.tensor.matmul(score, sdec[:, 2 * k, :], feat[:, 2 * k + 1, 0:TE],
                             start=False, stop=False)
            nc.tensor.matmul(score, sdec[:, 2 * k + 1, :], feat[:, 2 * k, 0:TE],
                             start=False, stop=last)

        # k=0: |omega_0 * args| stays within the Sin table's range: skip the
        # range reduction and read the projections straight from PSUM
        # (biases folded per-partition).
        nc.scalar.activation(feat[:, 0, 0:TE], ep, AF.Sin,
                             scale=OMEGA[0], bias=b0[:, 0:1])
        nc.scalar.activation(feat[:, 1, 0:TE], ep, AF.Sin,
                             scale=OMEGA[0], bias=b0[:, 1:2])
        nc.scalar.activation(feat[:, 0, TE:], dp, AF.Sin,
                             scale=OMEGA[0], bias=b0[:, 2:3])
        nc.scalar.activation(feat[:, 1, TE:], dp, AF.Sin,
                             scale=OMEGA[0], bias=b0[:, 3:4])
        # prioritize k=1's reduction on DVE so ACT never starves, then emit
        # the deferred k=0 extras (linear term + dec-side scalings).
        g1 = chain(1)
        nc.vector.tensor_copy(ap_r, ap[:, 0:TE])
        nc.tensor.matmul(score, cvrep, ap_r, start=True, stop=False)
        score_terms(0)
        g2 = chain(2)
        sins(1, g1)
        score_terms(1)
        g3 = chain(3)
        sins(2, g2)
        score_terms(2)
        g4 = chain(4)
        sins(3, g3)
        score_terms(3)
        sins(4, g4)
        score_terms(4, last=True)

        # ---- split softmax + pipelined context ----
        esc = const.tile([ROWS, TE], bf16, tag="esc")
        esum2 = work.tile([ROWS, 2], f32, tag="esum2", name="esum2")
        at = ps_p.tile([128, NT, 128], bf16, tag="at", name="at")
        escT = const.tile([128, NT, 128], bf16, tag="escT")
        ctx_ps = ps_s.tile([ROWS, DE], f32, tag="ctx", name="ctx_ps")
        for h in range(2):
            sl = slice(h * 256, (h + 1) * 256)
            nc.scalar.activation(esc[:, sl], score[:, sl], AF.Exp,
                                 accum_out=esum2[:, h:h + 1])
            for t in (2 * h, 2 * h + 1):
                nc.tensor.transpose(at[:, t, :], esc[:, t * 128:(t + 1) * 128],
                                    ident_b)
            nc.vector.tensor_copy(escT[:, 2 * h:2 * h + 2, :],
                                  at[:, 2 * h:2 * h + 2, :])
            for t in (2 * h, 2 * h + 1):
                nc.tensor.matmul(ctx_ps, escT[:, t, :], enc_nat[:, t, :],
                                 start=(t == 0), stop=(t == NT - 1),
                                 skip_group_check=True)
        esum = work.tile([ROWS, 1], f32, tag="esum", name="esum")
        nc.vector.scalar_tensor_tensor(esum, esum2[:, 0:1], 1.0, esum2[:, 1:2],
                                       op0=ALU.mult, op1=ALU.add)
        rinv = work.tile([ROWS, 1], f32, tag="rinv", name="rinv")
        nc.vector.reciprocal(rinv, esum)
        attn_sb = const.tile([ROWS, TE], f32, tag="attn_sb")
        nc.vector.tensor_scalar_mul(attn_sb, esc, rinv)
        nc.sync.dma_start(out=attn_d[:, :], in_=attn_sb)
        ctx_sb = const.tile([ROWS, DE], f32, tag="ctx_sb")
        nc.vector.tensor_scalar_mul(ctx_sb, ctx_ps, rinv)
        nc.sync.dma_start(out=ctx_d[:, :], in_=ctx_sb)

    nc.compile()
    return nc


def _get_nc():
    if "nc" not in _CACHE:
        _CACHE["nc"] = _build_program()
    return _CACHE["nc"]


def _install_ntff_hook():
    import sys
    import types

    if "antenv.axon_hooks" not in sys.modules:
        mod = types.ModuleType("antenv.axon_hooks")
        mod._hook = None
        mod.set_axon_ntff_profile_hook = lambda h: setattr(mod, "_hook", h)
        mod.get_axon_ntff_profile_hook = lambda: mod._hook
        sys.modules["antenv.axon_hooks"] = mod
        try:
            from trn_agent_boot.trn_boot import _ntff_profile_via_ctypes

            mod._hook = _ntff_profile_via_ctypes("/opt/axon/libaxon_pjrt.so")
        except Exception as e:
            print(f"ntff hook install failed: {e}")
    import concourse.bass_utils as bu

    bu.upload_artifacts = lambda tmpdir: "local://" + str(tmpdir)


def run(inputs, trace=False):
    import ml_dtypes
    from concourse.bass_utils import run_bass_kernel_spmd

    if trace:
        _install_ntff_hook()

    nc = _get_nc()
    bf = ml_dtypes.bfloat16
    enc = np.asarray(inputs["encoder_out"], dtype=np.float32).astype(bf)
    dec = np.asarray(inputs["decoder_out"], dtype=np.float32).astype(bf)
    w1 = np.ascontiguousarray(np.asarray(inputs["W1_w"], np.float32).astype(bf))
    w2 = np.ascontiguousarray(np.asarray(inputs["W2_w"], np.float32).astype(bf))
    vpack = np.ascontiguousarray(
        np.stack([np.asarray(inputs["V_w"], np.float32)[:, 0],
                  np.asarray(inputs["W1_b"], np.float32),
                  np.asarray(inputs["W2_b"], np.float32)], axis=1))

    in_maps = []
    for c in range(N_CORES):
        b, h = c // 2, c % 2
        in_maps.append(
            {
                "encT": np.ascontiguousarray(enc[b].T),
                "decT": np.ascontiguousarray(dec[b, h * ROWS:(h + 1) * ROWS].T),
                "enc": np.ascontiguousarray(enc[b]),
                "w1": w1,
                "w2": w2,
                "vpack": vpack,
            }
        )

    res = run_bass_kernel_spmd(nc, in_maps, list(range(N_CORES)), trace=trace)

    context = np.empty((B, TD, DE), np.float32)
    attn = np.empty((B, TD, TE), np.float32)
    for c in range(N_CORES):
        b, h = c // 2, c % 2
        context[b, h * ROWS:(h + 1) * ROWS] = res.results[c]["ctx"]
        attn[b, h * ROWS:(h + 1) * ROWS] = res.results[c]["attn"]
    return (context, attn), res


def kernel(**inputs):
    (context, attn), _ = run(inputs)
    return context, attn
